# revision 11
# baseline (speedup 1.0000x reference)
"""DecompGridv3 embedding lookup on 8 Trainium2 NeuronCores — v4.

HW finding that drives this design: random-row gathers are bound by a
per-SWDGE-queue descriptor wall (~8.3ns/row on one queue, any row size
256B-768B), and the wall scales with SWDGE queues (4 queues -> ~1.9ns/row,
~HBM rate). The only batched gather primitive that works on HW is
dma_gather (custom Q7 ucode) — int16 row indices, so every table must be
addressable within 32768 rows per gather instruction:

  * points are SORTED BY GRID Z-SLICE on the host; each core gets 16
    consecutive z-values, each z-value is one gather "window" whose rows
    fit int16 within a per-core grid slab (17 z-planes, passed per core).
  * plane02/plane12 (both keyed by the z-correlated y=fl5 coordinate) use
    one per-core 32768-row slab each; row-base is subtracted on device.
  * plane01 (x0,x1 — uncorrelated with the sort) uses a BLOCK table:
    one 768B row per (3y,2x)-cell block holding a 4x3 cell patch; the
    point's 2x2 quad is selected on-device with tent-function weights.
  * line0 is a 256-row table — direct int16 gather.

Slot space per core: 16 windows x 9216 slots (max z-bucket is ~8.5k for
uniform inputs; asserted on host). Pad slots replicate a real point of the
window and are dropped on the host after the run.

All gathers are dma_gather on 4 SWDGE queues (round-robin). DVE does the
interpolation math (fp16 2x), Pool does descriptor generation + two of the
final multiplies, ACT does coordinate scaling/abs, sync/HWDGE streams.
"""

import contextlib
import numpy as np

import concourse.bacc as bacc
import concourse.tile as tile
import concourse.mybir as mybir
from concourse.bass import AP
from concourse.bass_utils import run_bass_kernel_spmd

F32 = mybir.dt.float32
F16 = mybir.dt.float16
I32 = mybir.dt.int32
I16 = mybir.dt.int16
ALU = mybir.AluOpType
ACTF = mybir.ActivationFunctionType

NF = 32
D3 = 128
P2 = 384
L1 = 256
B = 1 << 20
NCORES = 8

ZPC = 16                    # z-values (windows) per core
WSLOT = 9216                # slots per window (max z-bucket + margin)
NTHIRD = 3                  # gather tiles per window
REPEAT = 1
NQ = 4                      # SWDGE queues

SLOTS = ZPC * WSLOT         # 147456 slots/core
SCOL = SLOTS // 128         # 1152 output cols
WCOL = WSLOT // 128         # 72 cols per window
TCOL = WCOL // NTHIRD       # 24 cols per gather tile
TSLOT = TCOL * 128          # 3072 idxs per gather
PREPW = 1                   # windows per prep block


def _ins0(ap: AP, pos: int, count: int) -> AP:
    dims = [list(d) for d in ap.ap]
    dims.insert(pos, [0, count])
    return AP(ap.tensor, ap.offset, dims)


def _apv(ap: AP, extra_off: int, dims) -> AP:
    return AP(ap.tensor, ap.offset + extra_off,
              [list(ap.ap[0])] + [list(d) for d in dims])


def build_bass(d3=D3, p2=P2, l1=L1, zpc=ZPC, wslot=WSLOT, repeat=REPEAT,
               ncores=NCORES, nq=NQ):
    d32 = d3 * d3
    gslab = (zpc + 1) * d32          # grid slab rows per core
    p2slab = min(2 * 16384, p2 * p2 + 1024)  # plane slab rows (<=32768)
    # p01 block grid: Y = y//3 in [0, ceil(p2/3)), X = x//2 in [0, p2/2)
    nby = -(-p2 // 3)                # 128 for p2=384
    nbx = p2 // 2                    # 192
    p01r = nby * nbx
    slots = zpc * wslot
    scol = slots // 128
    wcol = wslot // 128
    tcol = wcol // NTHIRD
    tslot = tcol * 128
    wwrap = wslot // 16              # wrapped idx cols per window

    nc = bacc.Bacc("TRN2", target_bir_lowering=False, debug=False,
                   num_devices=ncores, num_swdge_queues=nq)
    xin = nc.dram_tensor("xin", [128, scol * 4], F32, kind="ExternalInput")
    tabgs = nc.dram_tensor("tabgs", [gslab, NF * 8], F16,
                           kind="ExternalInput")
    tp02 = nc.dram_tensor("tp02", [p2slab, NF * 4], F16,
                          kind="ExternalInput")
    tp12 = nc.dram_tensor("tp12", [p2slab, NF * 4], F16,
                          kind="ExternalInput")
    tp01 = nc.dram_tensor("tp01", [p01r, NF * 12], F16,
                          kind="ExternalInput")
    tabl = nc.dram_tensor("tabl", [l1, NF * 4], F16, kind="ExternalInput")
    cbase = nc.dram_tensor("cbase", [128, 4], F32, kind="ExternalInput")
    iotac = nc.dram_tensor("iotac", [128, 8], F32, kind="ExternalInput")
    out = nc.dram_tensor("out", [128, scol * NF], F16, kind="ExternalOutput")

    qctr = [0]

    def nextq():
        q = qctr[0] % nq
        qctr[0] += 1
        return q

    with tile.TileContext(nc) as tc:
        with contextlib.ExitStack() as ctx:
            kp = ctx.enter_context(tc.tile_pool(name="kp", bufs=1))
            cp = ctx.enter_context(tc.tile_pool(name="cp", bufs=2))
            ip = ctx.enter_context(tc.tile_pool(name="ip", bufs=2))
            gp = ctx.enter_context(tc.tile_pool(name="gp", bufs=2))
            rp = ctx.enter_context(tc.tile_pool(name="rp", bufs=1))
            op = ctx.enter_context(tc.tile_pool(name="op", bufs=3))

            cb = kp.tile([128, 4], F32, tag="cb")
            nc.sync.dma_start(cb[:], cbase.ap())
            iot = kp.tile([128, 8], F32, tag="iot")
            nc.sync.dma_start(iot[:], iotac.ap())

            rep_ctx = (tc.For_i(0, repeat, 1) if repeat > 1
                       else contextlib.nullcontext())
            with rep_ctx:
              for blk in range(zpc // PREPW):
                P = PREPW * wcol                    # cols in prep block
                j0 = blk * P
                # ---------------- prep: coords, weights, rows
                xs = cp.tile([128, P, 4], F32, tag="xs")
                nc.sync.dma_start(
                    xs[:], xin.ap()[:, j0 * 4:(j0 + P) * 4]
                           .rearrange("p (j c) -> p j c", c=4))
                xyz = xs[:, :, 0:3].rearrange("p j k -> p k j")
                fv = cp.tile([128, 7, P], F32, tag="fv")
                nc.scalar.activation(fv[:, 0:3, :], xyz, ACTF.Copy,
                                     bias=0.5 * (d3 - 1),
                                     scale=0.5 * (d3 - 1))
                nc.scalar.activation(fv[:, 3:6, :], xyz, ACTF.Copy,
                                     bias=0.5 * (p2 - 1),
                                     scale=0.5 * (p2 - 1))
                nc.scalar.activation(
                    fv[:, 6:7, :],
                    xs[:, :, 3:4].rearrange("p j k -> p k j"),
                    ACTF.Copy, bias=0.0, scale=float(l1))
                ri = cp.tile([128, 7, P], I32, tag="ri")
                nc.vector.tensor_copy(ri[:], fv[:])
                fl = cp.tile([128, 7, P], F32, tag="fl")
                nc.vector.tensor_copy(fl[:], ri[:])
                m = cp.tile([128, 7, P], F32, tag="m")
                nc.vector.tensor_tensor(out=m[:], in0=fl[:], in1=fv[:],
                                        op=ALU.is_gt)
                nc.vector.tensor_sub(fl[:], fl[:], m[:])
                P7 = cp.tile([128, 7, P, 2], F16, tag="P7")
                nc.vector.tensor_tensor(
                    out=P7[:, :, :, 1:2].rearrange("p s j o -> p s (j o)"),
                    in0=fv[:], in1=fl[:], op=ALU.subtract)
                nc.scalar.activation(
                    P7[:, :, :, 0:1].rearrange("p s j o -> p s (j o)"),
                    P7[:, :, :, 1:2].rearrange("p s j o -> p s (j o)"),
                    ACTF.Copy, bias=1.0, scale=-1.0)

                def fls(s):
                    return fl[:, s:s + 1, :].rearrange("p s j -> p (s j)")

                def fvs(s):
                    return fv[:, s:s + 1, :].rearrange("p s j -> p (s j)")

                def pair(s):
                    return P7[:, s:s + 1, :, :].rearrange(
                        "p s j o -> p (s j) o")

                # grid corner weights w8 (c = dz*4+dy*2+dx)
                zy = cp.tile([128, P, 2, 2], F16, tag="zy")
                nc.vector.tensor_mul(zy[:],
                                     pair(2).to_broadcast([128, P, 2, 2]),
                                     _ins0(pair(1), 2, 2))
                w8 = cp.tile([128, P, 8], F16, tag="w8")
                nc.vector.tensor_mul(
                    w8[:].rearrange("p j (a b) -> p j a b", b=2),
                    zy[:].rearrange("p j a b -> p j (a b)")
                         .to_broadcast([128, P, 4, 2]),
                    _ins0(pair(0), 2, 4))
                # p02/p12 quad weights (c = dx*2+dy)
                wp = cp.tile([128, P, 2, 4], F16, tag="wp")
                for pl, (sx, sy) in enumerate(((3, 5), (4, 5))):
                    nc.vector.tensor_mul(
                        wp[:, :, pl:pl + 1, :]
                            .rearrange("p j o (a b) -> p j (o a) b", b=2),
                        pair(sx).to_broadcast([128, P, 2, 2]),
                        _ins0(pair(sy), 2, 2))

                # p01 block-local position + tent weights (negated pair)
                def fdiv(src_ap, scale, tag):
                    t = cp.tile([128, P], F32, tag=tag + "t")
                    nc.vector.tensor_scalar(out=t[:], in0=src_ap,
                                            scalar1=scale, scalar2=None,
                                            op0=ALU.mult)
                    tri = cp.tile([128, P], I32, tag=tag + "i")
                    nc.vector.tensor_copy(tri[:], t[:])
                    tf = cp.tile([128, P], F32, tag=tag + "f")
                    nc.vector.tensor_copy(tf[:], tri[:])
                    tm = cp.tile([128, P], F32, tag=tag + "m")
                    nc.vector.tensor_tensor(out=tm[:], in0=tf[:], in1=t[:],
                                            op=ALU.is_gt)
                    nc.vector.tensor_sub(tf[:], tf[:], tm[:])
                    return tf

                Yf = fdiv(fls(4), 1.0 / 3.0 + 1e-7, "Y")
                Xf = fdiv(fls(3), 0.5, "X")
                fyl = cp.tile([128, P], F32, tag="fyl")
                nc.vector.scalar_tensor_tensor(
                    out=fyl[:], in0=Yf[:], scalar=-3.0, in1=fvs(4),
                    op0=ALU.mult, op1=ALU.add)
                fxl = cp.tile([128, P], F32, tag="fxl")
                nc.vector.scalar_tensor_tensor(
                    out=fxl[:], in0=Xf[:], scalar=-2.0, in1=fvs(3),
                    op0=ALU.mult, op1=ALU.add)

                def tent(loc, n, icol, tag):
                    d = cp.tile([128, P, n], F32, tag=tag + "d")
                    nc.vector.tensor_tensor(
                        out=d[:], in0=_ins0(iot[:, icol:icol + n], 1, P),
                        in1=_ins0(loc[:], 2, n), op=ALU.subtract)
                    nc.scalar.activation(d[:], d[:], ACTF.Abs,
                                         bias=0.0, scale=1.0)
                    w = cp.tile([128, P, n], F16, tag=tag + "w")
                    nc.vector.tensor_scalar(out=w[:], in0=d[:],
                                            scalar1=1.0, scalar2=0.0,
                                            op0=ALU.subtract, op1=ALU.min)
                    return w

                ywt = tent(fyl, 4, 0, "yw")          # -(tent), f16
                xwt = tent(fxl, 3, 4, "xw")
                wsel = cp.tile([128, P, 12], F16, tag="wsel")
                nc.vector.tensor_mul(
                    wsel[:].rearrange("p j (a b) -> p j a b", b=3),
                    _ins0(ywt[:], 3, 3), _ins0(xwt[:], 2, 4))

                # ---- gather rows
                ga = cp.tile([128, P], F32, tag="ga")
                nc.vector.scalar_tensor_tensor(
                    out=ga[:], in0=fls(1), scalar=float(d3), in1=fls(0),
                    op0=ALU.mult, op1=ALU.add)
                gloc = cp.tile([128, P], F32, tag="gloc")
                nc.vector.scalar_tensor_tensor(
                    out=gloc[:], in0=fls(2), scalar=float(d32), in1=ga[:],
                    op0=ALU.mult, op1=ALU.add)
                nc.vector.tensor_sub(gloc[:], gloc[:],
                                     cb[:, 0:1].to_broadcast([128, P]))
                t3b = cp.tile([128, P], F32, tag="t3b")
                nc.vector.tensor_sub(t3b[:], fls(3),
                                     cb[:, 1:2].to_broadcast([128, P]))
                t4b = cp.tile([128, P], F32, tag="t4b")
                nc.vector.tensor_sub(t4b[:], fls(4),
                                     cb[:, 1:2].to_broadcast([128, P]))
                r02 = cp.tile([128, P], F32, tag="r02")
                nc.vector.scalar_tensor_tensor(
                    out=r02[:], in0=fls(5), scalar=float(p2), in1=t3b[:],
                    op0=ALU.mult, op1=ALU.add)
                r12 = cp.tile([128, P], F32, tag="r12")
                nc.vector.scalar_tensor_tensor(
                    out=r12[:], in0=fls(5), scalar=float(p2), in1=t4b[:],
                    op0=ALU.mult, op1=ALU.add)
                r01 = cp.tile([128, P], F32, tag="r01")
                nc.vector.scalar_tensor_tensor(
                    out=r01[:], in0=Yf[:], scalar=float(nbx), in1=Xf[:],
                    op0=ALU.mult, op1=ALU.add)

                for wrel in range(PREPW):
                    w = blk * PREPW + wrel
                    c0 = wrel * wcol
                    # idx128[p, stream, c] int16 for this window
                    idx128 = ip.tile([128, 5, wcol], I16, tag="idx128")

                    def cast_to(stream, src_ap, bias):
                        dst = idx128[:, stream, :]
                        if bias == 0.0:
                            nc.vector.tensor_copy(dst, src_ap)
                        else:
                            nc.vector.tensor_scalar(
                                out=dst, in0=src_ap, scalar1=bias,
                                scalar2=None, op0=ALU.add)

                    sl = slice(c0, c0 + wcol)
                    cast_to(0, gloc[:, sl], -float(w * d32))
                    cast_to(1, r02[:, sl], 0.0)
                    cast_to(2, r12[:, sl], 0.0)
                    cast_to(3, r01[:, sl], 0.0)
                    cast_to(4, fl[:, 6:7, sl].rearrange("p s j -> p (s j)"),
                            0.0)

                    # shuffle to wrapped layout + replicate to 128 parts
                    idxw = ip.tile([128, 5, wwrap], I16, tag="idxw")
                    for g in range(8):
                        nc.sync.dma_start(
                            _apv(idxw[0:16, :, :], g,
                                 [[wwrap, 5], [8, wcol]]),
                            idx128[16 * g:16 * (g + 1), :, :])
                    for grp in range(1, 8):
                        nc.sync.dma_start(idxw[16 * grp:16 * (grp + 1), :, :],
                                          idxw[0:16, :, :])

                    for t in range(NTHIRD):
                        tc0 = c0 + t * tcol            # col in prep block
                        oc0 = (w * wcol + t * tcol)    # col in out space
                        iw = slice(t * (tslot // 16), (t + 1) * (tslot // 16))
                        gg = gp.tile([128, tcol, NF * 8], F16, tag="gg")
                        nc.gpsimd.dma_gather(
                            out_ap=gg[:],
                            in_ap=tabgs.ap()[w * d32:(w + 2) * d32, :],
                            idxs_ap=idxw[:, 0, iw], num_idxs=tslot,
                            num_idxs_reg=tslot, elem_size=NF * 8,
                            single_packet=False, queue_num=nextq())
                        g02 = gp.tile([128, tcol, NF * 4], F16, tag="g02")
                        nc.gpsimd.dma_gather(
                            out_ap=g02[:], in_ap=tp02.ap(),
                            idxs_ap=idxw[:, 1, iw], num_idxs=tslot,
                            num_idxs_reg=tslot, elem_size=NF * 4,
                            single_packet=False, queue_num=nextq())
                        g12 = gp.tile([128, tcol, NF * 4], F16, tag="g12")
                        nc.gpsimd.dma_gather(
                            out_ap=g12[:], in_ap=tp12.ap(),
                            idxs_ap=idxw[:, 2, iw], num_idxs=tslot,
                            num_idxs_reg=tslot, elem_size=NF * 4,
                            single_packet=False, queue_num=nextq())
                        gb01 = gp.tile([128, tcol, NF * 12], F16, tag="gb01")
                        nc.gpsimd.dma_gather(
                            out_ap=gb01[:], in_ap=tp01.ap(),
                            idxs_ap=idxw[:, 3, iw], num_idxs=tslot,
                            num_idxs_reg=tslot, elem_size=NF * 12,
                            single_packet=False, queue_num=nextq())
                        ld = gp.tile([128, tcol, NF * 4], F16, tag="ld")
                        nc.gpsimd.dma_gather(
                            out_ap=ld[:], in_ap=tabl.ap(),
                            idxs_ap=idxw[:, 4, iw], num_idxs=tslot,
                            num_idxs_reg=tslot, elem_size=NF * 4,
                            single_packet=False, queue_num=nextq())

                        ts = slice(tc0, tc0 + tcol)
                        # grid: weight multiply + 8->1 reduction
                        gv = gg[:].rearrange("p j (f c) -> p j f c", c=8)
                        nc.vector.tensor_mul(
                            gv, gv, _ins0(w8[:, ts, :], 2, NF))
                        ggap = gg[:]
                        g4 = rp.tile([128, tcol, NF, 4], F16, tag="g4")
                        nc.vector.tensor_add(
                            g4[:],
                            _apv(ggap, 0, [[NF * 8, tcol], [8, NF], [1, 4]]),
                            _apv(ggap, 4, [[NF * 8, tcol], [8, NF], [1, 4]]))
                        g4ap = g4[:]
                        g2lo = _apv(g4ap, 0, [[NF * 4, tcol], [4, NF], [1, 2]])
                        nc.vector.tensor_add(
                            g2lo, g2lo,
                            _apv(g4ap, 2, [[NF * 4, tcol], [4, NF], [1, 2]]))
                        g1 = rp.tile([128, tcol, NF], F16, tag="g1")
                        nc.vector.tensor_add(
                            g1[:],
                            _apv(g4ap, 0, [[NF * 4, tcol], [4, NF]]),
                            _apv(g4ap, 1, [[NF * 4, tcol], [4, NF]]))
                        # p02 / p12: quad multiply + 4->1
                        q2t = rp.tile([128, tcol, 2, NF, 2], F16, tag="q2t")
                        qout = []
                        for pi, gq in enumerate((g02, g12)):
                            qv = gq[:].rearrange("p j (f c) -> p j f c", c=4)
                            nc.vector.tensor_mul(
                                qv, qv,
                                _ins0(wp[:, ts, pi, :], 2, NF))
                            gqap = gq[:]
                            q2 = q2t[:, :, pi, :, :]
                            nc.vector.tensor_add(
                                q2,
                                _apv(gqap, 0,
                                     [[NF * 4, tcol], [4, NF], [1, 2]]),
                                _apv(gqap, 2,
                                     [[NF * 4, tcol], [4, NF], [1, 2]]))
                            q1 = rp.tile([128, tcol, NF], F16,
                                         tag=f"q1_{pi}", name=f"q1_{pi}")
                            q2ap = q2t[:]
                            off = pi * NF * 2
                            nc.vector.tensor_add(
                                q1[:],
                                _apv(q2ap, off, [[NF * 4, tcol], [2, NF]]),
                                _apv(q2ap, off + 1,
                                     [[NF * 4, tcol], [2, NF]]))
                            qout.append(q1)
                        # p01 block: select-weights multiply + 12->1
                        bv = gb01[:].rearrange("p j (c f) -> p j c f", f=NF)
                        nc.vector.tensor_mul(
                            bv, bv, _ins0(wsel[:, ts, :], 3, NF))
                        bap = gb01[:]
                        b6 = rp.tile([128, tcol, 6, NF], F16, tag="b6")
                        nc.vector.tensor_add(
                            b6[:],
                            _apv(bap, 0,
                                 [[NF * 12, tcol], [NF, 6], [1, NF]]),
                            _apv(bap, 6 * NF,
                                 [[NF * 12, tcol], [NF, 6], [1, NF]]))
                        b6ap = b6[:]
                        b3lo = _apv(b6ap, 0,
                                    [[NF * 6, tcol], [NF, 3], [1, NF]])
                        nc.vector.tensor_add(
                            b3lo, b3lo,
                            _apv(b6ap, 3 * NF,
                                 [[NF * 6, tcol], [NF, 3], [1, NF]]))
                        q01 = rp.tile([128, tcol, NF], F16, tag="q01")
                        nc.vector.tensor_add(
                            q01[:],
                            _apv(b6ap, 0, [[NF * 6, tcol], [1, NF]]),
                            _apv(b6ap, NF, [[NF * 6, tcol], [1, NF]]))
                        nc.vector.tensor_add(
                            q01[:], q01[:],
                            _apv(b6ap, 2 * NF, [[NF * 6, tcol], [1, NF]]))
                        # line (row content: [f, (i, i+1)] in first NF*2 els)
                        ldap = ld[:]
                        lv = _apv(ldap, 0, [[NF * 4, tcol], [2, NF], [1, 2]])
                        wl = P7[:, 6:7, ts, :].rearrange("p s j o -> p (s j) o")
                        nc.vector.tensor_mul(lv, lv, _ins0(wl, 2, NF))
                        l1t = rp.tile([128, tcol, NF], F16, tag="l1t")
                        nc.vector.tensor_add(
                            l1t[:],
                            _apv(ldap, 0, [[NF * 4, tcol], [2, NF]]),
                            _apv(ldap, 1, [[NF * 4, tcol], [2, NF]]))
                        # final products (all DVE: Pool must stay in the
                        # dma_gather ucode library to avoid reload thrash)
                        t2 = rp.tile([128, tcol, NF], F16, tag="t2")
                        nc.vector.tensor_mul(t2[:], qout[0][:], qout[1][:])
                        t3 = rp.tile([128, tcol, NF], F16, tag="t3")
                        nc.vector.tensor_mul(t3[:], t2[:], q01[:])
                        t1 = rp.tile([128, tcol, NF], F16, tag="t1")
                        nc.vector.tensor_mul(t1[:], g1[:], l1t[:])
                        ot = op.tile([128, tcol, NF], F16, tag="ot")
                        nc.vector.tensor_mul(ot[:], t3[:], t1[:])
                        nc.sync.dma_start(
                            out.ap()[:, oc0 * NF:(oc0 + tcol) * NF],
                            ot[:].rearrange("p u f -> p (u f)"))

    nc.compile()
    return nc


# ---------------------------------------------------------------- host side

def _prep_tables(grid3d, plane01, plane02, plane12, line0,
                 d3=D3, p2=P2, l1=L1):
    f = grid3d.shape[0]
    gt = np.ascontiguousarray(
        grid3d.transpose(1, 2, 3, 0)).astype(np.float16)       # (z, y, x, f)
    gpad = np.empty((d3 + 1, d3 + 1, d3 + 1, f), np.float16)
    gpad[:d3, :d3, :d3] = gt
    gpad[d3, :d3, :d3] = gt[d3 - 1]
    gpad[:, d3, :d3] = gpad[:, d3 - 1, :d3]
    gpad[:, :, d3] = gpad[:, :, d3 - 1]
    del gt
    tabg = np.empty((d3, d3, d3, f, 8), np.float16)
    for dz in range(2):
        for dy in range(2):
            for dx in range(2):
                tabg[..., dz * 4 + dy * 2 + dx] = \
                    gpad[dz:dz + d3, dy:dy + d3, dx:dx + d3, :]
    del gpad
    tabg = tabg.reshape(d3 ** 3, f * 8)

    def quad(p):
        pt = np.ascontiguousarray(p.transpose(1, 2, 0)).astype(np.float16)
        ppad = np.empty((p2 + 1, p2 + 1, f), np.float16)
        ppad[:p2, :p2] = pt
        ppad[p2, :p2] = pt[p2 - 1]
        ppad[:, p2] = ppad[:, p2 - 1]
        t = np.empty((p2, p2, f, 4), np.float16)
        for dx in range(2):
            for dy in range(2):
                t[..., dx * 2 + dy] = ppad[dy:dy + p2, dx:dx + p2, :]
        return t.reshape(p2 * p2, f * 4)

    q02 = quad(plane02)
    q12 = quad(plane12)

    # plane01 block table: row (Y, X) = cells y in [3Y,3Y+4), x in [2X,2X+3)
    nby = -(-p2 // 3)
    nbx = p2 // 2
    pt = np.ascontiguousarray(plane01.transpose(1, 2, 0)).astype(np.float16)
    ppad = np.zeros((3 * nby + 4, 2 * nbx + 3, f), np.float16)
    ppad[:p2, :p2] = pt
    ppad[p2:p2 + 2, :p2] = pt[p2 - 1]
    ppad[:, p2:p2 + 2] = ppad[:, p2 - 1:p2]
    t01 = np.empty((nby, nbx, 4, 3, f), np.float16)
    for dy in range(4):
        for dx in range(3):
            t01[:, :, dy, dx, :] = \
                ppad[dy:dy + 3 * nby:3, dx:dx + 2 * nbx:2, :]
    tabp01b = t01.reshape(nby * nbx, 12 * f)

    lt = np.ascontiguousarray(line0.T).astype(np.float16)       # (l1, f)
    tl = np.empty((l1, f, 2), np.float16)
    tl[:, :, 0] = lt
    tl[:-1, :, 1] = lt[1:]
    tl[-1, :, 1] = lt[-1]
    tabl = np.zeros((l1, f * 4), np.float16)
    tabl[:, :f * 2] = tl.reshape(l1, f * 2)
    return tabg, q02, q12, tabp01b, tabl


def build_inmaps(x, tabg, q02, q12, tabp01b, tabl,
                 d3=D3, p2=P2, zpc=ZPC, wslot=WSLOT):
    """Sort points by grid z, build per-core slabs + layouts.
    Returns (in_maps, slotmaps)."""
    d32 = d3 * d3
    fl2 = np.floor((x[:, 2] + 1.0) * 0.5 * (d3 - 1)).astype(np.int64)
    order = np.argsort(fl2, kind="stable")
    fs = fl2[order]
    bounds = np.searchsorted(fs, np.arange(d3))   # start of each z-bucket
    bounds = np.append(bounds, len(x))
    iotac = np.zeros((128, 8), np.float32)
    iotac[:, 0:4] = np.arange(4)
    iotac[:, 4:7] = np.arange(3)

    in_maps, slotmaps = [], []
    for k in range(NCORES):
        zlo = zpc * k
        slots = zpc * wslot
        slotmap = np.full(slots, -1, np.int64)
        xk = np.zeros((slots, 4), np.float32)
        for w in range(zpc):
            z0 = zlo + w
            if z0 < d3 - 1:
                b0, b1 = bounds[z0], bounds[z0 + 1]
            else:
                b0 = b1 = 0
            n = b1 - b0
            assert n <= wslot, f"z-bucket {z0} has {n} > {wslot} points"
            s0 = w * wslot
            if n > 0:
                pts = order[b0:b1]
                slotmap[s0:s0 + n] = pts
                xk[s0:s0 + n] = x[pts]
                xk[s0 + n:s0 + wslot] = x[pts[0]]
            else:
                # synthetic point inside this window (never read back)
                xs = np.zeros(4, np.float32)
                xs[2] = (z0 + 0.5) / (0.5 * (d3 - 1)) - 1.0
                xk[s0:s0 + wslot] = xs
        scol = slots // 128
        xin = np.ascontiguousarray(
            xk.reshape(scol, 128, 4).transpose(1, 0, 2)
            .reshape(128, scol * 4))

        gs0 = zlo * d32
        gs1 = gs0 + (zpc + 1) * d32
        tabgs = np.zeros(((zpc + 1) * d32, tabg.shape[1]), np.float16)
        real = min(gs1, tabg.shape[0]) - gs0
        tabgs[:real] = tabg[gs0:gs0 + real]

        base5 = (zpc * k * (2 * p2 - 2)) // (2 * (d3 - 1))
        pbase = base5 * p2
        p2slab = min(2 * 16384, p2 * p2 + 1024)
        def slab(q):
            s = np.zeros((p2slab, q.shape[1]), np.float16)
            real = min(pbase + p2slab, q.shape[0]) - pbase
            s[:real] = q[pbase:pbase + real]
            return s

        cbase = np.zeros((128, 4), np.float32)
        cbase[:, 0] = zlo * d32
        cbase[:, 1] = pbase

        in_maps.append({"xin": xin, "tabgs": tabgs, "tp02": slab(q02),
                        "tp12": slab(q12), "tp01": tabp01b, "tabl": tabl,
                        "cbase": cbase, "iotac": iotac})
        slotmaps.append(slotmap)
    return in_maps, slotmaps


_NC_CACHE = {}


def kernel(x, grid3d, plane01, plane02, plane12, line0):
    x = np.asarray(x, np.float32)
    tabs = _prep_tables(
        np.asarray(grid3d, np.float32), np.asarray(plane01, np.float32),
        np.asarray(plane02, np.float32), np.asarray(plane12, np.float32),
        np.asarray(line0, np.float32))
    if "nc" not in _NC_CACHE:
        _NC_CACHE["nc"] = build_bass()
    nc = _NC_CACHE["nc"]

    in_maps, slotmaps = build_inmaps(x, *tabs)
    res = run_bass_kernel_spmd(nc, in_maps, core_ids=list(range(NCORES)))
    outfull = np.zeros((x.shape[0], NF), np.float32)
    for k in range(NCORES):
        o = res.results[k]["out"].astype(np.float32)
        o = o.reshape(128, SCOL, NF).transpose(1, 0, 2).reshape(SLOTS, NF)
        sm = slotmaps[k]
        valid = sm >= 0
        outfull[sm[valid]] = o[valid]
    return outfull
